# revision 11
# baseline (speedup 1.0000x reference)
"""Trainium2 Bass kernel for nn_MixedLinear_89979564851799.

The reference computes y = x @ W.T where W is the block-dequantized weight;
setup_inputs() ships the module's precomputed dequantized transposed weight
w_t (IN, OUT), so y == x @ w_t up to fp32 matmul reassociation.  The kernel
runs a single 8192x4096x4096 matmul, data-parallel over tokens across 8
NeuronCores.

Numerics (v3): mixed bf16 / fp8-DoubleRow.  The last K8 = 256*N8 of the
contraction runs as fp8e4 DoubleRow matmuls (2 k-tiles per MM at the same
216ns issue gap as one bf16 MM -> 2x throughput on that span; measured on
this part, probe_doublerow.py).  The fp8 range covers the module's
fp8-quantized weight partition (k in [3584,4096), whose dequantized values
are EXACTLY representable in TRN fp8e4 under a power-2 scale) plus
256*(N8-2) columns of the fp4 partition (e4m3 rounding error ~2.4% rms on
that slice).  x is e4m3 on the fp8 range.  CPU simulation of the exact
scheme on the reference data: rel err 9.6e-3 (N8=2) / 1.34e-2 (N8=3) /
1.63e-2 (N8=4) vs the 2e-2 gate; bf16-only measures 2.26e-3.

Scale handling: fp8 operands need power-2 scaling (x*2^a, w*2^b) to sit in
e4m3 range; the bf16 operands are pre-scaled by the same powers (exact in
bf16) so both matmul flavors accumulate into one PSUM group, and the
psum->sbuf copy applies 2^-(a+b) (tensor_scalar_mul, same cost as the
plain copy).

Schedule (v2, from the ntff profile of v1): interleaved per-k-tile DMA
descriptors; n-chunk 0 k-OUTER across 8 m-tiles / 8 psum banks (PE starts
~8us in, never starves: demand 222GB/s < ~300GB/s delivered); n-chunks 1-7
m-inner with staggered psum copies; activation table warmed at start; the
final group's copy/store split across engines/queues to shrink the tail.
"""

import os
import numpy as np
import ml_dtypes

P = 128
TOKENS, IN, OUT = 8192, 4096, 4096
NCORES = 8
M_PER_CORE = TOKENS // NCORES      # 1024
KT = IN // P                       # 32 k-tiles
MT = M_PER_CORE // P               # 8 m-tiles
NCH = 8                            # n chunks
NW = OUT // NCH                    # 512 cols per chunk (= 1 PSUM bank fp32)

N8 = 4                             # DoubleRow 256-k blocks (fp8 span = 256*N8)
KTB = KT - 2 * N8                  # bf16 k-tiles
KSPLIT = KTB * P                   # k index where the fp8 span starts
GS = KTB // 2                      # bf16 w chunk size (k-tiles) for nch 1-7

FP8_MAX = 240.0                    # TRN fp8e4 max normal

# Results of the traced run (exec_time_ns etc.) for test harnesses.
LAST_RESULT = None
_BUILT = {}


def _patch_tile_drain():
    """The walrus build in this container rejects instructions carrying more
    than one sync-wait (CoreV3GenImpl setupSyncWait: "Too many sync wait
    commands").  Tile's scheduler freely assigns several waits to one
    instruction, so (a) wrap _commit_instruction to hoist extra waits onto
    single-wait NOPs on the same engine just before the offender, and
    (b) split the kernel-tail Drain (which collects one wait per DMA queue)
    into a chain of single-wait Drains."""
    import concourse.tile as tile_mod
    import concourse.mybir as mybir
    import bass_rust
    from concourse.vector_clock import ScopedClock

    if getattr(tile_mod.TileContext, "_single_wait_drain_patch", False):
        return

    orig_commit = tile_mod.TileContext._commit_instruction

    def _commit_instruction(self, inst, lazy_reg_writes=True):
        si = getattr(inst, "sync_info", None)
        if (
            si is not None
            and len(si.on_wait) > 1
            and inst.engine != mybir.EngineType.Unassigned
        ):
            waits = list(si.on_wait)
            for w in waits[:-1]:
                nop = mybir.InstNoOp(
                    name=self.nc.get_next_instruction_name(),
                    engine=inst.engine,
                    sync_info=mybir.SyncInfo(on_wait=[w], on_update=[]),
                    bass_nofuse=True,
                )
                orig_commit(self, nop, lazy_reg_writes=False)
            inst.sync_info = mybir.SyncInfo(
                on_wait=[waits[-1]], on_update=list(si.on_update)
            )
        return orig_commit(self, inst, lazy_reg_writes)

    tile_mod.TileContext._commit_instruction = _commit_instruction

    def _drain_and_barrier(self, tick_clock, wait_clock):
        drain_inst = self.nc.sync.drain()
        wait_clock.add_sem_waits(
            drain_inst.ins, ScopedClock({None: tick_clock.global_clock})
        )
        si = drain_inst.ins.sync_info
        if si is not None and len(si.on_wait) > 1:
            waits = list(si.on_wait)
            drain_inst.ins.sync_info = bass_rust.SyncInfo(
                on_wait=[waits[0]], on_update=list(si.on_update)
            )
            for w in waits[1:]:
                extra = self.nc.sync.drain()
                extra.ins.sync_info = bass_rust.SyncInfo(on_wait=[w], on_update=[])
        self.nc.all_engine_barrier()
        popped = self.nc._tile_sem_poison_stack.pop()
        assert popped is self._sem_poison
        self.nc.clear_and_free_semaphores(list(self.sems.allocated().values()))
        self.nc.all_engine_barrier()

    tile_mod.TileContext._drain_and_barrier = _drain_and_barrier
    tile_mod.TileContext._single_wait_drain_patch = True


def _build(descale):
    """descale = 2^-(a+b), baked into the psum->sbuf copies."""
    if descale in _BUILT:
        return _BUILT[descale]
    import concourse.bass as bass
    import concourse.tile as tile
    from concourse import mybir

    _patch_tile_drain()

    nc = bass.Bass("TRN2", debug=False)
    xb_d = nc.dram_tensor(
        "xb", [KTB, P, M_PER_CORE], mybir.dt.bfloat16, kind="ExternalInput"
    ).ap()
    x8_d = nc.dram_tensor(
        "x8", [N8, P, 2, M_PER_CORE], mybir.dt.float8e4, kind="ExternalInput"
    ).ap()
    # n-chunk 0 of w, per-k-tile descriptors
    wb0_d = nc.dram_tensor(
        "wb0", [KTB, P, NW], mybir.dt.bfloat16, kind="ExternalInput"
    ).ap()
    w80_d = nc.dram_tensor(
        "w80", [N8, P, 2, NW], mybir.dt.float8e4, kind="ExternalInput"
    ).ap()
    # n-chunks 1-7: bf16 in two GS-k-tile chunks, fp8 in one block
    wbr_d = nc.dram_tensor(
        "wbr", [NCH - 1, 2, P, GS, NW], mybir.dt.bfloat16, kind="ExternalInput"
    ).ap()
    w8r_d = nc.dram_tensor(
        "w8r", [NCH - 1, P, N8, 2, NW], mybir.dt.float8e4, kind="ExternalInput"
    ).ap()
    y_d = nc.dram_tensor(
        "y", [M_PER_CORE, OUT], mybir.dt.float32, kind="ExternalOutput"
    ).ap()

    with tile.TileContext(nc) as tc:
        with (
            tc.tile_pool(name="xt", bufs=1) as xt_pool,
            tc.tile_pool(name="w0", bufs=1) as w0_pool,
            tc.tile_pool(name="wr", bufs=2) as wr_pool,
            tc.tile_pool(name="y", bufs=8) as y_pool,
            tc.tile_pool(name="ps", bufs=1, space="PSUM") as ps_pool,
        ):
            # Warm the activation engine's function table (1.3us, overlaps
            # the DMA head) so the tail's scalar.mul doesn't pay it.
            warm = xt_pool.tile([P, 2], mybir.dt.float32, name="warm")
            nc.scalar.mul(warm[:], warm[:], 0.0)
            # Warm the PE clock: HAM un-throttles (1.2->2.4GHz) only after
            # ~3.4us of sustained busy, so burn the ~11us DMA head on dummy
            # matmuls over never-written SBUF (values irrelevant; the real
            # groups start with start=True which resets the bank).
            dum_l = xt_pool.tile([P, P], mybir.dt.bfloat16, name="dum_l")
            dum_r = xt_pool.tile([P, NW], mybir.dt.bfloat16, name="dum_r")
            nc.gpsimd.memset(dum_l[:], 0)
            nc.gpsimd.memset(dum_r[:], 0)
            ps_warm = ps_pool.tile([P, NW], mybir.dt.float32, name="ps0_0")
            for _ in range(50):
                nc.tensor.matmul(
                    ps_warm[:], lhsT=dum_l[:], rhs=dum_r[:], start=True, stop=True
                )

            xb_sb = xt_pool.tile(
                [P, KTB, M_PER_CORE], mybir.dt.bfloat16, name="xb"
            )
            x8_sb = xt_pool.tile(
                [P, N8, 2, M_PER_CORE], mybir.dt.float8e4, name="x8"
            )
            # Head: interleave x-slice and w0 descriptors in consumption
            # order so MM(kt=0) waits on just the first two transfers.
            wb0_sbs = []
            for kt in range(KTB):
                nc.sync.dma_start(xb_sb[:, kt, :], xb_d[kt])
                w_sb = w0_pool.tile([P, NW], mybir.dt.bfloat16, name=f"wb0_{kt}")
                nc.sync.dma_start(w_sb[:], wb0_d[kt])
                wb0_sbs.append(w_sb)
            w80_sbs = []
            for blk in range(N8):
                nc.sync.dma_start(x8_sb[:, blk], x8_d[blk])
                w_sb = w0_pool.tile([P, 2, NW], mybir.dt.float8e4, name=f"w80_{blk}")
                nc.sync.dma_start(w_sb[:], w80_d[blk])
                w80_sbs.append(w_sb)
            # Prefetch stream for n-chunks 1-7 (pool slots throttle the
            # lookahead to ~1 chunk).
            wbr_sbs = {}
            w8r_sbs = {}
            for nch in range(1, NCH):
                for h in range(2):
                    w_sb = wr_pool.tile(
                        [P, GS, NW], mybir.dt.bfloat16, name=f"wbr{h}"
                    )
                    nc.sync.dma_start(w_sb[:], wbr_d[nch - 1, h])
                    wbr_sbs[(nch, h)] = w_sb
                w_sb = wr_pool.tile(
                    [P, N8, 2, NW], mybir.dt.float8e4, name="w8r"
                )
                nc.sync.dma_start(w_sb[:], w8r_d[nch - 1])
                w8r_sbs[nch] = w_sb

            def mm_group(ps, mt, wb_of_kt, w8_of_blk):
                msl = slice(mt * P, (mt + 1) * P)
                for kt in range(KTB):
                    nc.tensor.matmul(
                        ps[:],
                        lhsT=xb_sb[:, kt, msl],
                        rhs=wb_of_kt(kt),
                        start=(kt == 0),
                        stop=False,
                    )
                for blk in range(N8):
                    nc.tensor.matmul(
                        ps[:],
                        lhsT=x8_sb[:, blk, :, msl],
                        rhs=w8_of_blk(blk),
                        start=False,
                        stop=(blk == N8 - 1),
                        perf_mode=mybir.MatmulPerfMode.DoubleRow,
                    )

            def emit_out(mt, nch, ps, last):
                """psum -> sbuf (descale by 2^-(a+b)) -> DRAM.  The last
                n-chunk spreads stores across both hw DMA queues (the input
                queue is idle by then) and the final group also splits the
                copy across engines, shrinking the serial tail."""
                y_sb = y_pool.tile([P, NW], mybir.dt.float32, name="y_sb")
                half = NW // 2
                if last:
                    nc.vector.tensor_scalar_mul(y_sb[:, :half], ps[:, :half], descale)
                    nc.scalar.mul(y_sb[:, half:], ps[:, half:], descale)
                else:
                    nc.vector.tensor_scalar_mul(y_sb[:], ps[:], descale)
                nq = 4 if last else 2
                step = NW // nq
                engs = (nc.scalar, nc.sync) if nch == NCH - 1 else (nc.scalar,)
                for s in range(nq):
                    engs[s % len(engs)].dma_start(
                        y_d[
                            mt * P : (mt + 1) * P,
                            nch * NW + s * step : nch * NW + (s + 1) * step,
                        ],
                        y_sb[:, s * step : (s + 1) * step],
                    )

            # n-chunk 0: k-outer over all 8 m-tiles (8 psum banks live) so
            # each w tile feeds 8 back-to-back MMs while the next streams in.
            ps0 = [
                ps_pool.tile([P, NW], mybir.dt.float32, name=f"ps0_{m}")
                for m in range(MT)
            ]
            for kt in range(KTB):
                for mt in range(MT):
                    nc.tensor.matmul(
                        ps0[mt][:],
                        lhsT=xb_sb[:, kt, mt * P : (mt + 1) * P],
                        rhs=wb0_sbs[kt][:],
                        start=(kt == 0),
                        stop=False,
                    )
            for blk in range(N8):
                for mt in range(MT):
                    nc.tensor.matmul(
                        ps0[mt][:],
                        lhsT=x8_sb[:, blk, :, mt * P : (mt + 1) * P],
                        rhs=w80_sbs[blk][:],
                        start=False,
                        stop=(blk == N8 - 1),
                        perf_mode=mybir.MatmulPerfMode.DoubleRow,
                    )
            for mt in range(MT):
                emit_out(mt, 0, ps0[mt], last=False)

            # n-chunks 1-7: m-inner (psum copies stagger across the sweep).
            for nch in range(1, NCH):
                for mt in range(MT):
                    ps = ps_pool.tile([P, NW], mybir.dt.float32, name=f"ps0_{mt}")
                    mm_group(
                        ps,
                        mt,
                        lambda kt, n=nch: wbr_sbs[(n, kt // GS)][:, kt % GS, :],
                        lambda blk, n=nch: w8r_sbs[n][:, blk],
                    )
                    emit_out(
                        mt, nch, ps, last=(nch == NCH - 1 and mt == MT - 1)
                    )
    _BUILT[descale] = nc
    return nc


def kernel(x, w_q_fp4, w_os_fp4, w_is_fp4, w_t, w_q_fp8, w_s_fp8):
    global LAST_RESULT
    from concourse.bass_utils import run_bass_kernel_spmd

    x = np.asarray(x, dtype=np.float32)
    w_t = np.asarray(w_t, dtype=np.float32)

    bf16 = ml_dtypes.bfloat16
    e4m3 = ml_dtypes.float8_e4m3  # TRN fp8e4: max normal 240

    # power-2 scales placing the fp8-span operands in e4m3 range
    a = float(np.floor(np.log2(FP8_MAX / np.abs(x).max())))
    b = float(np.floor(np.log2(FP8_MAX / np.abs(w_t[KSPLIT:, :]).max())))
    sa, sb = 2.0**a, 2.0**b
    descale = float(2.0 ** (-(a + b)))

    nc = _build(descale)

    def to8(v, s):
        return np.clip(v * s, -FP8_MAX, FP8_MAX).astype(e4m3)

    xt = np.ascontiguousarray(x.T)                     # [IN, TOKENS] fp32
    xb_all = (xt[:KSPLIT] * sa).astype(bf16)           # [KSPLIT, TOKENS]
    x8_all = to8(xt[KSPLIT:], sa)                      # [2*N8*P, TOKENS]

    wsc = w_t * sb
    # n-chunk 0
    wb0 = np.ascontiguousarray(wsc[:KSPLIT, :NW]).astype(bf16).reshape(KTB, P, NW)
    w80 = np.ascontiguousarray(
        to8(wsc[KSPLIT:, :NW], 1.0).reshape(N8, 2, P, NW).transpose(0, 2, 1, 3)
    )
    # n-chunks 1-7
    wbr = np.ascontiguousarray(
        wsc[:KSPLIT, NW:]
        .astype(bf16)
        .reshape(2, GS, P, NCH - 1, NW)
        .transpose(3, 0, 2, 1, 4)
    )
    w8r = np.ascontiguousarray(
        to8(wsc[KSPLIT:, NW:], 1.0)
        .reshape(N8, 2, P, NCH - 1, NW)
        .transpose(3, 2, 0, 1, 4)
    )
    in_maps = []
    for i in range(NCORES):
        msl = slice(i * M_PER_CORE, (i + 1) * M_PER_CORE)
        xb = np.ascontiguousarray(xb_all[:, msl]).reshape(KTB, P, M_PER_CORE)
        x8 = np.ascontiguousarray(
            x8_all[:, msl].reshape(N8, 2, P, M_PER_CORE).transpose(0, 2, 1, 3)
        )
        in_maps.append(
            {"xb": xb, "x8": x8, "wb0": wb0, "w80": w80, "wbr": wbr, "w8r": w8r}
        )
    res = None
    for attempt in range(3):
        try:
            res = run_bass_kernel_spmd(
                nc,
                in_maps,
                list(range(NCORES)),
                trace=bool(os.environ.get("BASS_TRACE")),
            )
            break
        except Exception:
            # transient device errors (e.g. NRT_EXEC_UNIT_UNRECOVERABLE)
            # have been observed once and succeeded on retry
            if attempt == 2:
                raise
    LAST_RESULT = res
    return np.concatenate([res.results[i]["y"] for i in range(NCORES)], axis=0)


# revision 12
# speedup vs baseline: 1.0117x; 1.0117x over previous
"""Trainium2 Bass kernel for nn_MixedLinear_89979564851799.

The reference computes y = x @ W.T where W is the block-dequantized weight;
setup_inputs() ships the module's precomputed dequantized transposed weight
w_t (IN, OUT), so y == x @ w_t up to fp32 matmul reassociation.  The kernel
runs a single 8192x4096x4096 matmul, data-parallel over tokens across 8
NeuronCores.

Numerics (v3): mixed bf16 / fp8-DoubleRow.  The last K8 = 256*N8 of the
contraction runs as fp8e4 DoubleRow matmuls (2 k-tiles per MM at the same
216ns issue gap as one bf16 MM -> 2x throughput on that span; measured on
this part, probe_doublerow.py).  The fp8 range covers the module's
fp8-quantized weight partition (k in [3584,4096), whose dequantized values
are EXACTLY representable in TRN fp8e4 under a power-2 scale) plus
256*(N8-2) columns of the fp4 partition (e4m3 rounding error ~2.4% rms on
that slice).  x is e4m3 on the fp8 range.  CPU simulation of the exact
scheme on the reference data: rel err 9.6e-3 (N8=2) / 1.34e-2 (N8=3) /
1.63e-2 (N8=4) vs the 2e-2 gate; bf16-only measures 2.26e-3.

Scale handling: fp8 operands need power-2 scaling (x*2^a, w*2^b) to sit in
e4m3 range; the bf16 operands are pre-scaled by the same powers (exact in
bf16) so both matmul flavors accumulate into one PSUM group, and the
psum->sbuf copy applies 2^-(a+b) (tensor_scalar_mul, same cost as the
plain copy).

Schedule (v2, from the ntff profile of v1): interleaved per-k-tile DMA
descriptors; n-chunk 0 k-OUTER across 8 m-tiles / 8 psum banks (PE starts
~8us in, never starves: demand 222GB/s < ~300GB/s delivered); n-chunks 1-7
m-inner with staggered psum copies; activation table warmed at start; the
final group's copy/store split across engines/queues to shrink the tail.
"""

import os
import numpy as np
import ml_dtypes

P = 128
TOKENS, IN, OUT = 8192, 4096, 4096
NCORES = 8
M_PER_CORE = TOKENS // NCORES      # 1024
KT = IN // P                       # 32 k-tiles
MT = M_PER_CORE // P               # 8 m-tiles
NCH = 8                            # n chunks
NW = OUT // NCH                    # 512 cols per chunk (= 1 PSUM bank fp32)

N8 = 4                             # DoubleRow 256-k blocks (fp8 span = 256*N8)
KTB = KT - 2 * N8                  # bf16 k-tiles
KSPLIT = KTB * P                   # k index where the fp8 span starts
GS = KTB // 2                      # bf16 w chunk size (k-tiles) for nch 1-7

FP8_MAX = 240.0                    # TRN fp8e4 max normal

# Results of the traced run (exec_time_ns etc.) for test harnesses.
LAST_RESULT = None
_BUILT = {}


def _patch_tile_drain():
    """The walrus build in this container rejects instructions carrying more
    than one sync-wait (CoreV3GenImpl setupSyncWait: "Too many sync wait
    commands").  Tile's scheduler freely assigns several waits to one
    instruction, so (a) wrap _commit_instruction to hoist extra waits onto
    single-wait NOPs on the same engine just before the offender, and
    (b) split the kernel-tail Drain (which collects one wait per DMA queue)
    into a chain of single-wait Drains."""
    import concourse.tile as tile_mod
    import concourse.mybir as mybir
    import bass_rust
    from concourse.vector_clock import ScopedClock

    if getattr(tile_mod.TileContext, "_single_wait_drain_patch", False):
        return

    orig_commit = tile_mod.TileContext._commit_instruction

    def _commit_instruction(self, inst, lazy_reg_writes=True):
        si = getattr(inst, "sync_info", None)
        if (
            si is not None
            and len(si.on_wait) > 1
            and inst.engine != mybir.EngineType.Unassigned
        ):
            waits = list(si.on_wait)
            for w in waits[:-1]:
                nop = mybir.InstNoOp(
                    name=self.nc.get_next_instruction_name(),
                    engine=inst.engine,
                    sync_info=mybir.SyncInfo(on_wait=[w], on_update=[]),
                    bass_nofuse=True,
                )
                orig_commit(self, nop, lazy_reg_writes=False)
            inst.sync_info = mybir.SyncInfo(
                on_wait=[waits[-1]], on_update=list(si.on_update)
            )
        return orig_commit(self, inst, lazy_reg_writes)

    tile_mod.TileContext._commit_instruction = _commit_instruction

    def _drain_and_barrier(self, tick_clock, wait_clock):
        drain_inst = self.nc.sync.drain()
        wait_clock.add_sem_waits(
            drain_inst.ins, ScopedClock({None: tick_clock.global_clock})
        )
        si = drain_inst.ins.sync_info
        if si is not None and len(si.on_wait) > 1:
            waits = list(si.on_wait)
            drain_inst.ins.sync_info = bass_rust.SyncInfo(
                on_wait=[waits[0]], on_update=list(si.on_update)
            )
            for w in waits[1:]:
                extra = self.nc.sync.drain()
                extra.ins.sync_info = bass_rust.SyncInfo(on_wait=[w], on_update=[])
        self.nc.all_engine_barrier()
        popped = self.nc._tile_sem_poison_stack.pop()
        assert popped is self._sem_poison
        self.nc.clear_and_free_semaphores(list(self.sems.allocated().values()))
        self.nc.all_engine_barrier()

    tile_mod.TileContext._drain_and_barrier = _drain_and_barrier
    tile_mod.TileContext._single_wait_drain_patch = True


def _build(descale):
    """descale = 2^-(a+b), baked into the psum->sbuf copies."""
    if descale in _BUILT:
        return _BUILT[descale]
    import concourse.bass as bass
    import concourse.tile as tile
    from concourse import mybir

    _patch_tile_drain()

    nc = bass.Bass("TRN2", debug=False)
    xb_d = nc.dram_tensor(
        "xb", [KTB, P, M_PER_CORE], mybir.dt.bfloat16, kind="ExternalInput"
    ).ap()
    x8_d = nc.dram_tensor(
        "x8", [N8, P, 2, M_PER_CORE], mybir.dt.float8e4, kind="ExternalInput"
    ).ap()
    # n-chunk 0 of w, per-k-tile descriptors
    wb0_d = nc.dram_tensor(
        "wb0", [KTB, P, NW], mybir.dt.bfloat16, kind="ExternalInput"
    ).ap()
    w80_d = nc.dram_tensor(
        "w80", [N8, P, 2, NW], mybir.dt.float8e4, kind="ExternalInput"
    ).ap()
    # n-chunks 1-7: bf16 in two GS-k-tile chunks, fp8 in one block
    wbr_d = nc.dram_tensor(
        "wbr", [NCH - 1, 2, P, GS, NW], mybir.dt.bfloat16, kind="ExternalInput"
    ).ap()
    w8r_d = nc.dram_tensor(
        "w8r", [NCH - 1, P, N8, 2, NW], mybir.dt.float8e4, kind="ExternalInput"
    ).ap()
    y_d = nc.dram_tensor(
        "y", [M_PER_CORE, OUT], mybir.dt.float32, kind="ExternalOutput"
    ).ap()

    with tile.TileContext(nc) as tc:
        with (
            tc.tile_pool(name="xt", bufs=1) as xt_pool,
            tc.tile_pool(name="w0", bufs=1) as w0_pool,
            tc.tile_pool(name="wr", bufs=2) as wr_pool,
            tc.tile_pool(name="y", bufs=8) as y_pool,
            tc.tile_pool(name="ps", bufs=1, space="PSUM") as ps_pool,
        ):
            # Warm the activation engine's function table (1.3us, overlaps
            # the DMA head) so the tail's scalar.mul doesn't pay it.
            warm = xt_pool.tile([P, 2], mybir.dt.float32, name="warm")
            nc.scalar.mul(warm[:], warm[:], 0.0)
            # Warm the PE clock: HAM un-throttles (1.2->2.4GHz) only after
            # ~3.4us of sustained busy, so burn the ~11us DMA head on dummy
            # matmuls over never-written SBUF (values irrelevant; the real
            # groups start with start=True which resets the bank).
            dum_l = xt_pool.tile([P, P], mybir.dt.bfloat16, name="dum_l")
            dum_r = xt_pool.tile([P, NW], mybir.dt.bfloat16, name="dum_r")
            nc.vector.memset(dum_l[:], 0)
            nc.vector.memset(dum_r[:], 0)
            ps_warm = ps_pool.tile([P, NW], mybir.dt.float32, name="ps0_0")
            for _ in range(20):
                nc.tensor.matmul(
                    ps_warm[:], lhsT=dum_l[:], rhs=dum_r[:], start=True, stop=True
                )

            xb_sb = xt_pool.tile(
                [P, KTB, M_PER_CORE], mybir.dt.bfloat16, name="xb"
            )
            x8_sb = xt_pool.tile(
                [P, N8, 2, M_PER_CORE], mybir.dt.float8e4, name="x8"
            )
            # Head: interleave x-slice and w0 descriptors in consumption
            # order so MM(kt=0) waits on just the first two transfers.
            wb0_sbs = []
            for kt in range(KTB):
                nc.sync.dma_start(xb_sb[:, kt, :], xb_d[kt])
                w_sb = w0_pool.tile([P, NW], mybir.dt.bfloat16, name=f"wb0_{kt}")
                nc.sync.dma_start(w_sb[:], wb0_d[kt])
                wb0_sbs.append(w_sb)
            w80_sbs = []
            for blk in range(N8):
                nc.sync.dma_start(x8_sb[:, blk], x8_d[blk])
                w_sb = w0_pool.tile([P, 2, NW], mybir.dt.float8e4, name=f"w80_{blk}")
                nc.sync.dma_start(w_sb[:], w80_d[blk])
                w80_sbs.append(w_sb)
            # Prefetch stream for n-chunks 1-7 (pool slots throttle the
            # lookahead to ~1 chunk).
            wbr_sbs = {}
            w8r_sbs = {}
            for nch in range(1, NCH):
                for h in range(2):
                    w_sb = wr_pool.tile(
                        [P, GS, NW], mybir.dt.bfloat16, name=f"wbr{h}"
                    )
                    nc.sync.dma_start(w_sb[:], wbr_d[nch - 1, h])
                    wbr_sbs[(nch, h)] = w_sb
                w_sb = wr_pool.tile(
                    [P, N8, 2, NW], mybir.dt.float8e4, name="w8r"
                )
                nc.sync.dma_start(w_sb[:], w8r_d[nch - 1])
                w8r_sbs[nch] = w_sb

            def mm_group(ps, mt, wb_of_kt, w8_of_blk):
                msl = slice(mt * P, (mt + 1) * P)
                for kt in range(KTB):
                    nc.tensor.matmul(
                        ps[:],
                        lhsT=xb_sb[:, kt, msl],
                        rhs=wb_of_kt(kt),
                        start=(kt == 0),
                        stop=False,
                    )
                for blk in range(N8):
                    nc.tensor.matmul(
                        ps[:],
                        lhsT=x8_sb[:, blk, :, msl],
                        rhs=w8_of_blk(blk),
                        start=False,
                        stop=(blk == N8 - 1),
                        perf_mode=mybir.MatmulPerfMode.DoubleRow,
                    )

            def emit_out(mt, nch, ps, last):
                """psum -> sbuf (descale by 2^-(a+b)) -> DRAM.  The last
                n-chunk spreads stores across both hw DMA queues (the input
                queue is idle by then) and the final group also splits the
                copy across engines, shrinking the serial tail."""
                y_sb = y_pool.tile([P, NW], mybir.dt.float32, name="y_sb")
                half = NW // 2
                if last:
                    nc.vector.tensor_scalar_mul(y_sb[:, :half], ps[:, :half], descale)
                    nc.scalar.mul(y_sb[:, half:], ps[:, half:], descale)
                else:
                    nc.vector.tensor_scalar_mul(y_sb[:], ps[:], descale)
                nq = 4 if last else 2
                step = NW // nq
                engs = (nc.scalar, nc.sync) if nch == NCH - 1 else (nc.scalar,)
                for s in range(nq):
                    engs[s % len(engs)].dma_start(
                        y_d[
                            mt * P : (mt + 1) * P,
                            nch * NW + s * step : nch * NW + (s + 1) * step,
                        ],
                        y_sb[:, s * step : (s + 1) * step],
                    )

            # n-chunk 0: k-outer over all 8 m-tiles (8 psum banks live) so
            # each w tile feeds 8 back-to-back MMs while the next streams in.
            ps0 = [
                ps_pool.tile([P, NW], mybir.dt.float32, name=f"ps0_{m}")
                for m in range(MT)
            ]
            for kt in range(KTB):
                for mt in range(MT):
                    nc.tensor.matmul(
                        ps0[mt][:],
                        lhsT=xb_sb[:, kt, mt * P : (mt + 1) * P],
                        rhs=wb0_sbs[kt][:],
                        start=(kt == 0),
                        stop=False,
                    )
            for blk in range(N8):
                for mt in range(MT):
                    nc.tensor.matmul(
                        ps0[mt][:],
                        lhsT=x8_sb[:, blk, :, mt * P : (mt + 1) * P],
                        rhs=w80_sbs[blk][:],
                        start=False,
                        stop=(blk == N8 - 1),
                        perf_mode=mybir.MatmulPerfMode.DoubleRow,
                    )
            for mt in range(MT):
                emit_out(mt, 0, ps0[mt], last=False)

            # n-chunks 1-7: m-inner (psum copies stagger across the sweep).
            for nch in range(1, NCH):
                for mt in range(MT):
                    ps = ps_pool.tile([P, NW], mybir.dt.float32, name=f"ps0_{mt}")
                    mm_group(
                        ps,
                        mt,
                        lambda kt, n=nch: wbr_sbs[(n, kt // GS)][:, kt % GS, :],
                        lambda blk, n=nch: w8r_sbs[n][:, blk],
                    )
                    emit_out(
                        mt, nch, ps, last=(nch == NCH - 1 and mt == MT - 1)
                    )
    _BUILT[descale] = nc
    return nc


def kernel(x, w_q_fp4, w_os_fp4, w_is_fp4, w_t, w_q_fp8, w_s_fp8):
    global LAST_RESULT
    from concourse.bass_utils import run_bass_kernel_spmd

    x = np.asarray(x, dtype=np.float32)
    w_t = np.asarray(w_t, dtype=np.float32)

    bf16 = ml_dtypes.bfloat16
    e4m3 = ml_dtypes.float8_e4m3  # TRN fp8e4: max normal 240

    # power-2 scales placing the fp8-span operands in e4m3 range
    a = float(np.floor(np.log2(FP8_MAX / np.abs(x).max())))
    b = float(np.floor(np.log2(FP8_MAX / np.abs(w_t[KSPLIT:, :]).max())))
    sa, sb = 2.0**a, 2.0**b
    descale = float(2.0 ** (-(a + b)))

    nc = _build(descale)

    def to8(v, s):
        return np.clip(v * s, -FP8_MAX, FP8_MAX).astype(e4m3)

    xt = np.ascontiguousarray(x.T)                     # [IN, TOKENS] fp32
    xb_all = (xt[:KSPLIT] * sa).astype(bf16)           # [KSPLIT, TOKENS]
    x8_all = to8(xt[KSPLIT:], sa)                      # [2*N8*P, TOKENS]

    wsc = w_t * sb
    # n-chunk 0
    wb0 = np.ascontiguousarray(wsc[:KSPLIT, :NW]).astype(bf16).reshape(KTB, P, NW)
    w80 = np.ascontiguousarray(
        to8(wsc[KSPLIT:, :NW], 1.0).reshape(N8, 2, P, NW).transpose(0, 2, 1, 3)
    )
    # n-chunks 1-7
    wbr = np.ascontiguousarray(
        wsc[:KSPLIT, NW:]
        .astype(bf16)
        .reshape(2, GS, P, NCH - 1, NW)
        .transpose(3, 0, 2, 1, 4)
    )
    w8r = np.ascontiguousarray(
        to8(wsc[KSPLIT:, NW:], 1.0)
        .reshape(N8, 2, P, NCH - 1, NW)
        .transpose(3, 2, 0, 1, 4)
    )
    in_maps = []
    for i in range(NCORES):
        msl = slice(i * M_PER_CORE, (i + 1) * M_PER_CORE)
        xb = np.ascontiguousarray(xb_all[:, msl]).reshape(KTB, P, M_PER_CORE)
        x8 = np.ascontiguousarray(
            x8_all[:, msl].reshape(N8, 2, P, M_PER_CORE).transpose(0, 2, 1, 3)
        )
        in_maps.append(
            {"xb": xb, "x8": x8, "wb0": wb0, "w80": w80, "wbr": wbr, "w8r": w8r}
        )
    res = None
    for attempt in range(3):
        try:
            res = run_bass_kernel_spmd(
                nc,
                in_maps,
                list(range(NCORES)),
                trace=bool(os.environ.get("BASS_TRACE")),
            )
            break
        except Exception:
            # transient device errors (e.g. NRT_EXEC_UNIT_UNRECOVERABLE)
            # have been observed once and succeeded on retry
            if attempt == 2:
                raise
    LAST_RESULT = res
    return np.concatenate([res.results[i]["y"] for i in range(NCORES)], axis=0)


# revision 14
# speedup vs baseline: 1.0158x; 1.0041x over previous
"""Trainium2 Bass kernel for nn_MixedLinear_89979564851799.

The reference computes y = x @ W.T where W is the block-dequantized weight;
setup_inputs() ships the module's precomputed dequantized transposed weight
w_t (IN, OUT), so y == x @ w_t up to fp32 matmul reassociation.  The kernel
runs a single 8192x4096x4096 matmul, data-parallel over tokens across 8
NeuronCores.

Numerics (v3): mixed bf16 / fp8-DoubleRow.  The last K8 = 256*N8 of the
contraction runs as fp8e4 DoubleRow matmuls (2 k-tiles per MM at the same
216ns issue gap as one bf16 MM -> 2x throughput on that span; measured on
this part, probe_doublerow.py).  The fp8 range covers the module's
fp8-quantized weight partition (k in [3584,4096), whose dequantized values
are EXACTLY representable in TRN fp8e4 under a power-2 scale) plus
256*(N8-2) columns of the fp4 partition (e4m3 rounding error ~2.4% rms on
that slice).  x is e4m3 on the fp8 range.  CPU simulation of the exact
scheme on the reference data: rel err 9.6e-3 (N8=2) / 1.34e-2 (N8=3) /
1.63e-2 (N8=4) vs the 2e-2 gate; bf16-only measures 2.26e-3.

Scale handling: fp8 operands need power-2 scaling (x*2^a, w*2^b) to sit in
e4m3 range; the bf16 operands are pre-scaled by the same powers (exact in
bf16) so both matmul flavors accumulate into one PSUM group, and the
psum->sbuf copy applies 2^-(a+b) (tensor_scalar_mul, same cost as the
plain copy).

Schedule (v2, from the ntff profile of v1): interleaved per-k-tile DMA
descriptors; n-chunk 0 k-OUTER across 8 m-tiles / 8 psum banks (PE starts
~8us in, never starves: demand 222GB/s < ~300GB/s delivered); n-chunks 1-7
m-inner with staggered psum copies; activation table warmed at start; the
final group's copy/store split across engines/queues to shrink the tail.
"""

import os
import numpy as np
import ml_dtypes

P = 128
TOKENS, IN, OUT = 8192, 4096, 4096
NCORES = 8
M_PER_CORE = TOKENS // NCORES      # 1024
KT = IN // P                       # 32 k-tiles
MT = M_PER_CORE // P               # 8 m-tiles
NCH = 8                            # n chunks
NW = OUT // NCH                    # 512 cols per chunk (= 1 PSUM bank fp32)

N8 = 4                             # DoubleRow 256-k blocks (fp8 span = 256*N8)
KTB = KT - 2 * N8                  # bf16 k-tiles
KSPLIT = KTB * P                   # k index where the fp8 span starts
GS = KTB // 2                      # bf16 w chunk size (k-tiles) for nch 1-7

FP8_MAX = 240.0                    # TRN fp8e4 max normal

# Results of the traced run (exec_time_ns etc.) for test harnesses.
LAST_RESULT = None
_BUILT = {}


def _patch_tile_drain():
    """The walrus build in this container rejects instructions carrying more
    than one sync-wait (CoreV3GenImpl setupSyncWait: "Too many sync wait
    commands").  Tile's scheduler freely assigns several waits to one
    instruction, so (a) wrap _commit_instruction to hoist extra waits onto
    single-wait NOPs on the same engine just before the offender, and
    (b) split the kernel-tail Drain (which collects one wait per DMA queue)
    into a chain of single-wait Drains."""
    import concourse.tile as tile_mod
    import concourse.mybir as mybir
    import bass_rust
    from concourse.vector_clock import ScopedClock

    if getattr(tile_mod.TileContext, "_single_wait_drain_patch", False):
        return

    orig_commit = tile_mod.TileContext._commit_instruction

    def _commit_instruction(self, inst, lazy_reg_writes=True):
        si = getattr(inst, "sync_info", None)
        if (
            si is not None
            and len(si.on_wait) > 1
            and inst.engine != mybir.EngineType.Unassigned
        ):
            waits = list(si.on_wait)
            for w in waits[:-1]:
                nop = mybir.InstNoOp(
                    name=self.nc.get_next_instruction_name(),
                    engine=inst.engine,
                    sync_info=mybir.SyncInfo(on_wait=[w], on_update=[]),
                    bass_nofuse=True,
                )
                orig_commit(self, nop, lazy_reg_writes=False)
            inst.sync_info = mybir.SyncInfo(
                on_wait=[waits[-1]], on_update=list(si.on_update)
            )
        return orig_commit(self, inst, lazy_reg_writes)

    tile_mod.TileContext._commit_instruction = _commit_instruction

    def _drain_and_barrier(self, tick_clock, wait_clock):
        drain_inst = self.nc.sync.drain()
        wait_clock.add_sem_waits(
            drain_inst.ins, ScopedClock({None: tick_clock.global_clock})
        )
        si = drain_inst.ins.sync_info
        if si is not None and len(si.on_wait) > 1:
            waits = list(si.on_wait)
            drain_inst.ins.sync_info = bass_rust.SyncInfo(
                on_wait=[waits[0]], on_update=list(si.on_update)
            )
            for w in waits[1:]:
                extra = self.nc.sync.drain()
                extra.ins.sync_info = bass_rust.SyncInfo(on_wait=[w], on_update=[])
        self.nc.all_engine_barrier()
        popped = self.nc._tile_sem_poison_stack.pop()
        assert popped is self._sem_poison
        self.nc.clear_and_free_semaphores(list(self.sems.allocated().values()))
        self.nc.all_engine_barrier()

    tile_mod.TileContext._drain_and_barrier = _drain_and_barrier
    tile_mod.TileContext._single_wait_drain_patch = True


def _build(descale):
    """descale = 2^-(a+b), baked into the psum->sbuf copies."""
    if descale in _BUILT:
        return _BUILT[descale]
    import concourse.bass as bass
    import concourse.tile as tile
    from concourse import mybir

    _patch_tile_drain()

    nc = bass.Bass("TRN2", debug=False)
    xb_d = nc.dram_tensor(
        "xb", [KTB, P, M_PER_CORE], mybir.dt.bfloat16, kind="ExternalInput"
    ).ap()
    x8_d = nc.dram_tensor(
        "x8", [N8, P, 2, M_PER_CORE], mybir.dt.float8e4, kind="ExternalInput"
    ).ap()
    # n-chunk 0 of w, per-k-tile descriptors
    wb0_d = nc.dram_tensor(
        "wb0", [KTB, P, NW], mybir.dt.bfloat16, kind="ExternalInput"
    ).ap()
    w80_d = nc.dram_tensor(
        "w80", [N8, P, 2, NW], mybir.dt.float8e4, kind="ExternalInput"
    ).ap()
    # n-chunks 1-7: bf16 in two GS-k-tile chunks, fp8 in one block
    wbr_d = nc.dram_tensor(
        "wbr", [NCH - 1, 2, P, GS, NW], mybir.dt.bfloat16, kind="ExternalInput"
    ).ap()
    w8r_d = nc.dram_tensor(
        "w8r", [NCH - 1, P, N8, 2, NW], mybir.dt.float8e4, kind="ExternalInput"
    ).ap()
    y_d = nc.dram_tensor(
        "y", [M_PER_CORE, OUT], mybir.dt.float32, kind="ExternalOutput"
    ).ap()

    with tile.TileContext(nc) as tc:
        with (
            tc.tile_pool(name="xt", bufs=1) as xt_pool,
            tc.tile_pool(name="w0", bufs=1) as w0_pool,
            tc.tile_pool(name="wr", bufs=2) as wr_pool,
            tc.tile_pool(name="y", bufs=8) as y_pool,
            tc.tile_pool(name="ps", bufs=1, space="PSUM") as ps_pool,
        ):
            # Warm the activation engine's function table (1.3us, overlaps
            # the DMA head) so the tail's scalar.mul doesn't pay it.
            warm = xt_pool.tile([P, 2], mybir.dt.float32, name="warm")
            nc.scalar.mul(warm[:], warm[:], 0.0)
            # Warm the PE clock: HAM un-throttles (1.2->2.4GHz) only after
            # ~3.4us of sustained busy, so burn the ~11us DMA head on dummy
            # matmuls over never-written SBUF (values irrelevant; the real
            # groups start with start=True which resets the bank).
            dum_l = xt_pool.tile([P, P], mybir.dt.bfloat16, name="dum_l")
            dum_r = xt_pool.tile([P, NW], mybir.dt.bfloat16, name="dum_r")
            nc.vector.memset(dum_l[:], 0)
            nc.vector.memset(dum_r[:], 0)
            # Small-N dummies (~107ns cold) give fine-grained padding: they
            # keep the PE busy (HAM warm-up) from ~8.5us until the first
            # real operands land ~10.5us, with ~0.1us quantization.
            ps_warm = ps_pool.tile([P, NW], mybir.dt.float32, name="ps0_0")
            for _ in range(22):
                nc.tensor.matmul(
                    ps_warm[:, :64], lhsT=dum_l[:], rhs=dum_r[:, :64],
                    start=True, stop=True,
                )

            xb_sb = xt_pool.tile(
                [P, KTB, M_PER_CORE], mybir.dt.bfloat16, name="xb"
            )
            x8_sb = xt_pool.tile(
                [P, N8, 2, M_PER_CORE], mybir.dt.float8e4, name="x8"
            )
            # Head: interleave x-slice and w0 descriptors in consumption
            # order so MM(kt=0) waits on just the first two transfers.
            wb0_sbs = []
            for kt in range(KTB):
                if kt == 0:
                    # split so the first MM (kt0, m0) waits on 128KB, not 256KB
                    hm = M_PER_CORE // 2
                    nc.sync.dma_start(xb_sb[:, 0, :hm], xb_d[0, :, :hm])
                    w_sb = w0_pool.tile([P, NW], mybir.dt.bfloat16, name="wb0_0")
                    nc.sync.dma_start(w_sb[:], wb0_d[0])
                    nc.sync.dma_start(xb_sb[:, 0, hm:], xb_d[0, :, hm:])
                else:
                    nc.sync.dma_start(xb_sb[:, kt, :], xb_d[kt])
                    w_sb = w0_pool.tile([P, NW], mybir.dt.bfloat16, name=f"wb0_{kt}")
                    nc.sync.dma_start(w_sb[:], wb0_d[kt])
                wb0_sbs.append(w_sb)
            w80_sbs = []
            for blk in range(N8):
                nc.sync.dma_start(x8_sb[:, blk], x8_d[blk])
                w_sb = w0_pool.tile([P, 2, NW], mybir.dt.float8e4, name=f"w80_{blk}")
                nc.sync.dma_start(w_sb[:], w80_d[blk])
                w80_sbs.append(w_sb)
            # Prefetch stream for n-chunks 1-7 (pool slots throttle the
            # lookahead to ~1 chunk).
            wbr_sbs = {}
            w8r_sbs = {}
            for nch in range(1, NCH):
                for h in range(2):
                    w_sb = wr_pool.tile(
                        [P, GS, NW], mybir.dt.bfloat16, name=f"wbr{h}"
                    )
                    nc.sync.dma_start(w_sb[:], wbr_d[nch - 1, h])
                    wbr_sbs[(nch, h)] = w_sb
                w_sb = wr_pool.tile(
                    [P, N8, 2, NW], mybir.dt.float8e4, name="w8r"
                )
                nc.sync.dma_start(w_sb[:], w8r_d[nch - 1])
                w8r_sbs[nch] = w_sb

            def mm_group(ps, mt, wb_of_kt, w8_of_blk):
                msl = slice(mt * P, (mt + 1) * P)
                for kt in range(KTB):
                    nc.tensor.matmul(
                        ps[:],
                        lhsT=xb_sb[:, kt, msl],
                        rhs=wb_of_kt(kt),
                        start=(kt == 0),
                        stop=False,
                    )
                for blk in range(N8):
                    nc.tensor.matmul(
                        ps[:],
                        lhsT=x8_sb[:, blk, :, msl],
                        rhs=w8_of_blk(blk),
                        start=False,
                        stop=(blk == N8 - 1),
                        perf_mode=mybir.MatmulPerfMode.DoubleRow,
                    )

            def emit_out(mt, nch, ps, last):
                """psum -> sbuf (descale by 2^-(a+b)) -> DRAM.  The last
                n-chunk spreads stores across both hw DMA queues (the input
                queue is idle by then) and the final group also splits the
                copy across engines, shrinking the serial tail."""
                y_sb = y_pool.tile([P, NW], mybir.dt.float32, name="y_sb")
                half = NW // 2
                if last:
                    nc.vector.tensor_scalar_mul(y_sb[:, :half], ps[:, :half], descale)
                    nc.scalar.mul(y_sb[:, half:], ps[:, half:], descale)
                else:
                    nc.vector.tensor_scalar_mul(y_sb[:], ps[:], descale)
                nq = 4 if last else 2
                step = NW // nq
                engs = (nc.scalar, nc.sync) if nch == NCH - 1 else (nc.scalar,)
                for s in range(nq):
                    engs[s % len(engs)].dma_start(
                        y_d[
                            mt * P : (mt + 1) * P,
                            nch * NW + s * step : nch * NW + (s + 1) * step,
                        ],
                        y_sb[:, s * step : (s + 1) * step],
                    )

            # n-chunk 0: k-outer over all 8 m-tiles (8 psum banks live) so
            # each w tile feeds 8 back-to-back MMs while the next streams in.
            ps0 = [
                ps_pool.tile([P, NW], mybir.dt.float32, name=f"ps0_{m}")
                for m in range(MT)
            ]
            for kt in range(KTB):
                for mt in range(MT):
                    nc.tensor.matmul(
                        ps0[mt][:],
                        lhsT=xb_sb[:, kt, mt * P : (mt + 1) * P],
                        rhs=wb0_sbs[kt][:],
                        start=(kt == 0),
                        stop=False,
                    )
            for blk in range(N8):
                for mt in range(MT):
                    nc.tensor.matmul(
                        ps0[mt][:],
                        lhsT=x8_sb[:, blk, :, mt * P : (mt + 1) * P],
                        rhs=w80_sbs[blk][:],
                        start=False,
                        stop=(blk == N8 - 1),
                        perf_mode=mybir.MatmulPerfMode.DoubleRow,
                    )
            for mt in range(MT):
                emit_out(mt, 0, ps0[mt], last=False)

            # n-chunks 1-7: m-inner (psum copies stagger across the sweep).
            for nch in range(1, NCH):
                for mt in range(MT):
                    ps = ps_pool.tile([P, NW], mybir.dt.float32, name=f"ps0_{mt}")
                    mm_group(
                        ps,
                        mt,
                        lambda kt, n=nch: wbr_sbs[(n, kt // GS)][:, kt % GS, :],
                        lambda blk, n=nch: w8r_sbs[n][:, blk],
                    )
                    emit_out(
                        mt, nch, ps, last=(nch == NCH - 1 and mt == MT - 1)
                    )
    _BUILT[descale] = nc
    return nc


def kernel(x, w_q_fp4, w_os_fp4, w_is_fp4, w_t, w_q_fp8, w_s_fp8):
    global LAST_RESULT
    from concourse.bass_utils import run_bass_kernel_spmd

    x = np.asarray(x, dtype=np.float32)
    w_t = np.asarray(w_t, dtype=np.float32)

    bf16 = ml_dtypes.bfloat16
    e4m3 = ml_dtypes.float8_e4m3  # TRN fp8e4: max normal 240

    # power-2 scales placing the fp8-span operands in e4m3 range
    a = float(np.floor(np.log2(FP8_MAX / np.abs(x).max())))
    b = float(np.floor(np.log2(FP8_MAX / np.abs(w_t[KSPLIT:, :]).max())))
    sa, sb = 2.0**a, 2.0**b
    descale = float(2.0 ** (-(a + b)))

    nc = _build(descale)

    def to8(v, s):
        return np.clip(v * s, -FP8_MAX, FP8_MAX).astype(e4m3)

    xt = np.ascontiguousarray(x.T)                     # [IN, TOKENS] fp32
    xb_all = (xt[:KSPLIT] * sa).astype(bf16)           # [KSPLIT, TOKENS]
    x8_all = to8(xt[KSPLIT:], sa)                      # [2*N8*P, TOKENS]

    wsc = w_t * sb
    # n-chunk 0
    wb0 = np.ascontiguousarray(wsc[:KSPLIT, :NW]).astype(bf16).reshape(KTB, P, NW)
    w80 = np.ascontiguousarray(
        to8(wsc[KSPLIT:, :NW], 1.0).reshape(N8, 2, P, NW).transpose(0, 2, 1, 3)
    )
    # n-chunks 1-7
    wbr = np.ascontiguousarray(
        wsc[:KSPLIT, NW:]
        .astype(bf16)
        .reshape(2, GS, P, NCH - 1, NW)
        .transpose(3, 0, 2, 1, 4)
    )
    w8r = np.ascontiguousarray(
        to8(wsc[KSPLIT:, NW:], 1.0)
        .reshape(N8, 2, P, NCH - 1, NW)
        .transpose(3, 2, 0, 1, 4)
    )
    in_maps = []
    for i in range(NCORES):
        msl = slice(i * M_PER_CORE, (i + 1) * M_PER_CORE)
        xb = np.ascontiguousarray(xb_all[:, msl]).reshape(KTB, P, M_PER_CORE)
        x8 = np.ascontiguousarray(
            x8_all[:, msl].reshape(N8, 2, P, M_PER_CORE).transpose(0, 2, 1, 3)
        )
        in_maps.append(
            {"xb": xb, "x8": x8, "wb0": wb0, "w80": w80, "wbr": wbr, "w8r": w8r}
        )
    res = None
    for attempt in range(3):
        try:
            res = run_bass_kernel_spmd(
                nc,
                in_maps,
                list(range(NCORES)),
                trace=bool(os.environ.get("BASS_TRACE")),
            )
            break
        except Exception:
            # transient device errors (e.g. NRT_EXEC_UNIT_UNRECOVERABLE)
            # have been observed once and succeeded on retry
            if attempt == 2:
                raise
    LAST_RESULT = res
    return np.concatenate([res.results[i]["y"] for i in range(NCORES)], axis=0)


# revision 15
# speedup vs baseline: 1.0194x; 1.0035x over previous
"""Trainium2 Bass kernel for nn_MixedLinear_89979564851799.

The reference computes y = x @ W.T where W is the block-dequantized weight;
setup_inputs() ships the module's precomputed dequantized transposed weight
w_t (IN, OUT), so y == x @ w_t up to fp32 matmul reassociation.  The kernel
runs a single 8192x4096x4096 matmul, data-parallel over tokens across 8
NeuronCores.

Numerics (v3): mixed bf16 / fp8-DoubleRow.  The last K8 = 256*N8 of the
contraction runs as fp8e4 DoubleRow matmuls (2 k-tiles per MM at the same
216ns issue gap as one bf16 MM -> 2x throughput on that span; measured on
this part, probe_doublerow.py).  The fp8 range covers the module's
fp8-quantized weight partition (k in [3584,4096), whose dequantized values
are EXACTLY representable in TRN fp8e4 under a power-2 scale) plus
256*(N8-2) columns of the fp4 partition (e4m3 rounding error ~2.4% rms on
that slice).  x is e4m3 on the fp8 range.  CPU simulation of the exact
scheme on the reference data: rel err 9.6e-3 (N8=2) / 1.34e-2 (N8=3) /
1.63e-2 (N8=4) vs the 2e-2 gate; bf16-only measures 2.26e-3.

Scale handling: fp8 operands need power-2 scaling (x*2^a, w*2^b) to sit in
e4m3 range; the bf16 operands are pre-scaled by the same powers (exact in
bf16) so both matmul flavors accumulate into one PSUM group, and the
psum->sbuf copy applies 2^-(a+b) (tensor_scalar_mul, same cost as the
plain copy).

Schedule (v2, from the ntff profile of v1): interleaved per-k-tile DMA
descriptors; n-chunk 0 k-OUTER across 8 m-tiles / 8 psum banks (PE starts
~8us in, never starves: demand 222GB/s < ~300GB/s delivered); n-chunks 1-7
m-inner with staggered psum copies; activation table warmed at start; the
final group's copy/store split across engines/queues to shrink the tail.
"""

import os
import numpy as np
import ml_dtypes

P = 128
TOKENS, IN, OUT = 8192, 4096, 4096
NCORES = 8
M_PER_CORE = TOKENS // NCORES      # 1024
KT = IN // P                       # 32 k-tiles
MT = M_PER_CORE // P               # 8 m-tiles
NCH = 8                            # n chunks
NW = OUT // NCH                    # 512 cols per chunk (= 1 PSUM bank fp32)

N8 = 4                             # DoubleRow 256-k blocks (fp8 span = 256*N8)
KTB = KT - 2 * N8                  # bf16 k-tiles
KSPLIT = KTB * P                   # k index where the fp8 span starts
GS = KTB // 2                      # bf16 w chunk size (k-tiles) for nch 1-7

FP8_MAX = 240.0                    # TRN fp8e4 max normal

# Results of the traced run (exec_time_ns etc.) for test harnesses.
LAST_RESULT = None
_BUILT = {}


def _patch_tile_drain():
    """The walrus build in this container rejects instructions carrying more
    than one sync-wait (CoreV3GenImpl setupSyncWait: "Too many sync wait
    commands").  Tile's scheduler freely assigns several waits to one
    instruction, so (a) wrap _commit_instruction to hoist extra waits onto
    single-wait NOPs on the same engine just before the offender, and
    (b) split the kernel-tail Drain (which collects one wait per DMA queue)
    into a chain of single-wait Drains."""
    import concourse.tile as tile_mod
    import concourse.mybir as mybir
    import bass_rust
    from concourse.vector_clock import ScopedClock

    if getattr(tile_mod.TileContext, "_single_wait_drain_patch", False):
        return

    orig_commit = tile_mod.TileContext._commit_instruction

    def _commit_instruction(self, inst, lazy_reg_writes=True):
        si = getattr(inst, "sync_info", None)
        if (
            si is not None
            and len(si.on_wait) > 1
            and inst.engine != mybir.EngineType.Unassigned
        ):
            waits = list(si.on_wait)
            for w in waits[:-1]:
                nop = mybir.InstNoOp(
                    name=self.nc.get_next_instruction_name(),
                    engine=inst.engine,
                    sync_info=mybir.SyncInfo(on_wait=[w], on_update=[]),
                    bass_nofuse=True,
                )
                orig_commit(self, nop, lazy_reg_writes=False)
            inst.sync_info = mybir.SyncInfo(
                on_wait=[waits[-1]], on_update=list(si.on_update)
            )
        return orig_commit(self, inst, lazy_reg_writes)

    tile_mod.TileContext._commit_instruction = _commit_instruction

    def _drain_and_barrier(self, tick_clock, wait_clock):
        drain_inst = self.nc.sync.drain()
        wait_clock.add_sem_waits(
            drain_inst.ins, ScopedClock({None: tick_clock.global_clock})
        )
        si = drain_inst.ins.sync_info
        if si is not None and len(si.on_wait) > 1:
            waits = list(si.on_wait)
            drain_inst.ins.sync_info = bass_rust.SyncInfo(
                on_wait=[waits[0]], on_update=list(si.on_update)
            )
            for w in waits[1:]:
                extra = self.nc.sync.drain()
                extra.ins.sync_info = bass_rust.SyncInfo(on_wait=[w], on_update=[])
        self.nc.all_engine_barrier()
        popped = self.nc._tile_sem_poison_stack.pop()
        assert popped is self._sem_poison
        self.nc.clear_and_free_semaphores(list(self.sems.allocated().values()))
        self.nc.all_engine_barrier()

    tile_mod.TileContext._drain_and_barrier = _drain_and_barrier
    tile_mod.TileContext._single_wait_drain_patch = True


def _build(descale):
    """descale = 2^-(a+b), baked into the psum->sbuf copies."""
    if descale in _BUILT:
        return _BUILT[descale]
    import concourse.bass as bass
    import concourse.tile as tile
    from concourse import mybir

    _patch_tile_drain()

    nc = bass.Bass("TRN2", debug=False)
    xb_d = nc.dram_tensor(
        "xb", [KTB, P, M_PER_CORE], mybir.dt.bfloat16, kind="ExternalInput"
    ).ap()
    x8_d = nc.dram_tensor(
        "x8", [N8, P, 2, M_PER_CORE], mybir.dt.float8e4, kind="ExternalInput"
    ).ap()
    # n-chunk 0 of w, per-k-tile descriptors
    wb0_d = nc.dram_tensor(
        "wb0", [KTB, P, NW], mybir.dt.bfloat16, kind="ExternalInput"
    ).ap()
    w80_d = nc.dram_tensor(
        "w80", [N8, P, 2, NW], mybir.dt.float8e4, kind="ExternalInput"
    ).ap()
    # n-chunks 1-7: bf16 in two GS-k-tile chunks, fp8 in one block
    wbr_d = nc.dram_tensor(
        "wbr", [NCH - 1, 2, P, GS, NW], mybir.dt.bfloat16, kind="ExternalInput"
    ).ap()
    w8r_d = nc.dram_tensor(
        "w8r", [NCH - 1, P, N8, 2, NW], mybir.dt.float8e4, kind="ExternalInput"
    ).ap()
    y_d = nc.dram_tensor(
        "y", [M_PER_CORE, OUT], mybir.dt.float32, kind="ExternalOutput"
    ).ap()

    with tile.TileContext(nc) as tc:
        with (
            tc.tile_pool(name="xt", bufs=1) as xt_pool,
            tc.tile_pool(name="w0", bufs=1) as w0_pool,
            tc.tile_pool(name="wr", bufs=2) as wr_pool,
            tc.tile_pool(name="y", bufs=8) as y_pool,
            tc.tile_pool(name="ps", bufs=1, space="PSUM") as ps_pool,
        ):
            # Warm the activation engine's function table (1.3us, overlaps
            # the DMA head) so the tail's scalar.mul doesn't pay it.
            warm = xt_pool.tile([P, 2], mybir.dt.float32, name="warm")
            nc.scalar.mul(warm[:], warm[:], 0.0)
            # Warm the PE clock: HAM un-throttles (1.2->2.4GHz) only after
            # ~3.4us of sustained busy, so burn the ~11us DMA head on dummy
            # matmuls over never-written SBUF (values irrelevant; the real
            # groups start with start=True which resets the bank).
            dum_l = xt_pool.tile([P, P], mybir.dt.bfloat16, name="dum_l")
            dum_r = xt_pool.tile([P, NW], mybir.dt.bfloat16, name="dum_r")
            nc.vector.memset(dum_l[:], 0)
            nc.vector.memset(dum_r[:], 0)
            # Small-N dummies (~107ns cold) give fine-grained padding: they
            # keep the PE busy (HAM warm-up) from ~8.5us until the first
            # real operands land ~10.5us, with ~0.1us quantization.
            ps_warm = ps_pool.tile([P, NW], mybir.dt.float32, name="ps0_0")
            for _ in range(60):
                nc.tensor.matmul(
                    ps_warm[:, :64], lhsT=dum_l[:], rhs=dum_r[:, :64],
                    start=True, stop=True,
                )

            xb_sb = xt_pool.tile(
                [P, KTB, M_PER_CORE], mybir.dt.bfloat16, name="xb"
            )
            x8_sb = xt_pool.tile(
                [P, N8, 2, M_PER_CORE], mybir.dt.float8e4, name="x8"
            )
            # Head: interleave x-slice and w0 descriptors in consumption
            # order so MM(kt=0) waits on just the first two transfers.
            wb0_sbs = []
            for kt in range(KTB):
                if kt == 0:
                    # split so the first MM (kt0, m0) waits on 128KB, not 256KB
                    hm = M_PER_CORE // 2
                    nc.sync.dma_start(xb_sb[:, 0, :hm], xb_d[0, :, :hm])
                    w_sb = w0_pool.tile([P, NW], mybir.dt.bfloat16, name="wb0_0")
                    nc.sync.dma_start(w_sb[:], wb0_d[0])
                    nc.sync.dma_start(xb_sb[:, 0, hm:], xb_d[0, :, hm:])
                else:
                    nc.sync.dma_start(xb_sb[:, kt, :], xb_d[kt])
                    w_sb = w0_pool.tile([P, NW], mybir.dt.bfloat16, name=f"wb0_{kt}")
                    nc.sync.dma_start(w_sb[:], wb0_d[kt])
                wb0_sbs.append(w_sb)
            w80_sbs = []
            for blk in range(N8):
                nc.sync.dma_start(x8_sb[:, blk], x8_d[blk])
                w_sb = w0_pool.tile([P, 2, NW], mybir.dt.float8e4, name=f"w80_{blk}")
                nc.sync.dma_start(w_sb[:], w80_d[blk])
                w80_sbs.append(w_sb)
            # Prefetch stream for n-chunks 1-7 (pool slots throttle the
            # lookahead to ~1 chunk).
            wbr_sbs = {}
            w8r_sbs = {}
            for nch in range(1, NCH):
                for h in range(2):
                    w_sb = wr_pool.tile(
                        [P, GS, NW], mybir.dt.bfloat16, name=f"wbr{h}"
                    )
                    nc.sync.dma_start(w_sb[:], wbr_d[nch - 1, h])
                    wbr_sbs[(nch, h)] = w_sb
                w_sb = wr_pool.tile(
                    [P, N8, 2, NW], mybir.dt.float8e4, name="w8r"
                )
                nc.sync.dma_start(w_sb[:], w8r_d[nch - 1])
                w8r_sbs[nch] = w_sb

            def mm_group(ps, mt, wb_of_kt, w8_of_blk):
                msl = slice(mt * P, (mt + 1) * P)
                for kt in range(KTB):
                    nc.tensor.matmul(
                        ps[:],
                        lhsT=xb_sb[:, kt, msl],
                        rhs=wb_of_kt(kt),
                        start=(kt == 0),
                        stop=False,
                    )
                for blk in range(N8):
                    nc.tensor.matmul(
                        ps[:],
                        lhsT=x8_sb[:, blk, :, msl],
                        rhs=w8_of_blk(blk),
                        start=False,
                        stop=(blk == N8 - 1),
                        perf_mode=mybir.MatmulPerfMode.DoubleRow,
                    )

            def emit_out(mt, nch, ps, last):
                """psum -> sbuf (descale by 2^-(a+b)) -> DRAM.  The last
                n-chunk spreads stores across both hw DMA queues (the input
                queue is idle by then) and the final group also splits the
                copy across engines, shrinking the serial tail."""
                y_sb = y_pool.tile([P, NW], mybir.dt.float32, name="y_sb")
                half = NW // 2
                if last:
                    nc.vector.tensor_scalar_mul(y_sb[:, :half], ps[:, :half], descale)
                    nc.scalar.mul(y_sb[:, half:], ps[:, half:], descale)
                else:
                    nc.vector.tensor_scalar_mul(y_sb[:], ps[:], descale)
                nq = 4 if last else 2
                step = NW // nq
                engs = (nc.scalar, nc.sync) if nch == NCH - 1 else (nc.scalar,)
                for s in range(nq):
                    engs[s % len(engs)].dma_start(
                        y_d[
                            mt * P : (mt + 1) * P,
                            nch * NW + s * step : nch * NW + (s + 1) * step,
                        ],
                        y_sb[:, s * step : (s + 1) * step],
                    )

            # n-chunk 0: k-outer over all 8 m-tiles (8 psum banks live) so
            # each w tile feeds 8 back-to-back MMs while the next streams in.
            ps0 = [
                ps_pool.tile([P, NW], mybir.dt.float32, name=f"ps0_{m}")
                for m in range(MT)
            ]
            for kt in range(KTB):
                for mt in range(MT):
                    nc.tensor.matmul(
                        ps0[mt][:],
                        lhsT=xb_sb[:, kt, mt * P : (mt + 1) * P],
                        rhs=wb0_sbs[kt][:],
                        start=(kt == 0),
                        stop=False,
                    )
            for blk in range(N8):
                for mt in range(MT):
                    nc.tensor.matmul(
                        ps0[mt][:],
                        lhsT=x8_sb[:, blk, :, mt * P : (mt + 1) * P],
                        rhs=w80_sbs[blk][:],
                        start=False,
                        stop=(blk == N8 - 1),
                        perf_mode=mybir.MatmulPerfMode.DoubleRow,
                    )
            for mt in range(MT):
                emit_out(mt, 0, ps0[mt], last=False)

            # n-chunks 1-7: m-inner (psum copies stagger across the sweep).
            for nch in range(1, NCH):
                for mt in range(MT):
                    ps = ps_pool.tile([P, NW], mybir.dt.float32, name=f"ps0_{mt}")
                    mm_group(
                        ps,
                        mt,
                        lambda kt, n=nch: wbr_sbs[(n, kt // GS)][:, kt % GS, :],
                        lambda blk, n=nch: w8r_sbs[n][:, blk],
                    )
                    emit_out(
                        mt, nch, ps, last=(nch == NCH - 1 and mt == MT - 1)
                    )
    _BUILT[descale] = nc
    return nc


def kernel(x, w_q_fp4, w_os_fp4, w_is_fp4, w_t, w_q_fp8, w_s_fp8):
    global LAST_RESULT
    from concourse.bass_utils import run_bass_kernel_spmd

    x = np.asarray(x, dtype=np.float32)
    w_t = np.asarray(w_t, dtype=np.float32)

    bf16 = ml_dtypes.bfloat16
    e4m3 = ml_dtypes.float8_e4m3  # TRN fp8e4: max normal 240

    # power-2 scales placing the fp8-span operands in e4m3 range
    a = float(np.floor(np.log2(FP8_MAX / np.abs(x).max())))
    b = float(np.floor(np.log2(FP8_MAX / np.abs(w_t[KSPLIT:, :]).max())))
    sa, sb = 2.0**a, 2.0**b
    descale = float(2.0 ** (-(a + b)))

    nc = _build(descale)

    def to8(v, s):
        return np.clip(v * s, -FP8_MAX, FP8_MAX).astype(e4m3)

    xt = np.ascontiguousarray(x.T)                     # [IN, TOKENS] fp32
    xb_all = (xt[:KSPLIT] * sa).astype(bf16)           # [KSPLIT, TOKENS]
    x8_all = to8(xt[KSPLIT:], sa)                      # [2*N8*P, TOKENS]

    wsc = w_t * sb
    # n-chunk 0
    wb0 = np.ascontiguousarray(wsc[:KSPLIT, :NW]).astype(bf16).reshape(KTB, P, NW)
    w80 = np.ascontiguousarray(
        to8(wsc[KSPLIT:, :NW], 1.0).reshape(N8, 2, P, NW).transpose(0, 2, 1, 3)
    )
    # n-chunks 1-7
    wbr = np.ascontiguousarray(
        wsc[:KSPLIT, NW:]
        .astype(bf16)
        .reshape(2, GS, P, NCH - 1, NW)
        .transpose(3, 0, 2, 1, 4)
    )
    w8r = np.ascontiguousarray(
        to8(wsc[KSPLIT:, NW:], 1.0)
        .reshape(N8, 2, P, NCH - 1, NW)
        .transpose(3, 2, 0, 1, 4)
    )
    in_maps = []
    for i in range(NCORES):
        msl = slice(i * M_PER_CORE, (i + 1) * M_PER_CORE)
        xb = np.ascontiguousarray(xb_all[:, msl]).reshape(KTB, P, M_PER_CORE)
        x8 = np.ascontiguousarray(
            x8_all[:, msl].reshape(N8, 2, P, M_PER_CORE).transpose(0, 2, 1, 3)
        )
        in_maps.append(
            {"xb": xb, "x8": x8, "wb0": wb0, "w80": w80, "wbr": wbr, "w8r": w8r}
        )
    res = None
    for attempt in range(3):
        try:
            res = run_bass_kernel_spmd(
                nc,
                in_maps,
                list(range(NCORES)),
                trace=bool(os.environ.get("BASS_TRACE")),
            )
            break
        except Exception:
            # transient device errors (e.g. NRT_EXEC_UNIT_UNRECOVERABLE)
            # have been observed once and succeeded on retry
            if attempt == 2:
                raise
    LAST_RESULT = res
    return np.concatenate([res.results[i]["y"] for i in range(NCORES)], axis=0)


# revision 17
# speedup vs baseline: 1.0200x; 1.0006x over previous
"""Trainium2 Bass kernel for nn_MixedLinear_89979564851799.

The reference computes y = x @ W.T where W is the block-dequantized weight;
setup_inputs() ships the module's precomputed dequantized transposed weight
w_t (IN, OUT), so y == x @ w_t up to fp32 matmul reassociation.  The kernel
runs a single 8192x4096x4096 matmul, data-parallel over tokens across 8
NeuronCores.

Numerics (v3): mixed bf16 / fp8-DoubleRow.  The last K8 = 256*N8 of the
contraction runs as fp8e4 DoubleRow matmuls (2 k-tiles per MM at the same
216ns issue gap as one bf16 MM -> 2x throughput on that span; measured on
this part, probe_doublerow.py).  The fp8 range covers the module's
fp8-quantized weight partition (k in [3584,4096), whose dequantized values
are EXACTLY representable in TRN fp8e4 under a power-2 scale) plus
256*(N8-2) columns of the fp4 partition (e4m3 rounding error ~2.4% rms on
that slice).  x is e4m3 on the fp8 range.  CPU simulation of the exact
scheme on the reference data: rel err 9.6e-3 (N8=2) / 1.34e-2 (N8=3) /
1.63e-2 (N8=4) vs the 2e-2 gate; bf16-only measures 2.26e-3.

Scale handling: fp8 operands need power-2 scaling (x*2^a, w*2^b) to sit in
e4m3 range; the bf16 operands are pre-scaled by the same powers (exact in
bf16) so both matmul flavors accumulate into one PSUM group, and the
psum->sbuf copy applies 2^-(a+b) (tensor_scalar_mul, same cost as the
plain copy).

Schedule (v2, from the ntff profile of v1): interleaved per-k-tile DMA
descriptors; n-chunk 0 k-OUTER across 8 m-tiles / 8 psum banks (PE starts
~8us in, never starves: demand 222GB/s < ~300GB/s delivered); n-chunks 1-7
m-inner with staggered psum copies; activation table warmed at start; the
final group's copy/store split across engines/queues to shrink the tail.
"""

import os
import numpy as np
import ml_dtypes

P = 128
TOKENS, IN, OUT = 8192, 4096, 4096
NCORES = 8
M_PER_CORE = TOKENS // NCORES      # 1024
KT = IN // P                       # 32 k-tiles
MT = M_PER_CORE // P               # 8 m-tiles
NCH = 8                            # n chunks
NW = OUT // NCH                    # 512 cols per chunk (= 1 PSUM bank fp32)

N8 = 4                             # DoubleRow 256-k blocks (fp8 span = 256*N8)
KTB = KT - 2 * N8                  # bf16 k-tiles
KSPLIT = KTB * P                   # k index where the fp8 span starts
GS = KTB // 2                      # bf16 w chunk size (k-tiles) for nch 1-7

FP8_MAX = 240.0                    # TRN fp8e4 max normal

# Results of the traced run (exec_time_ns etc.) for test harnesses.
LAST_RESULT = None
_BUILT = {}


def _patch_tile_drain():
    """The walrus build in this container rejects instructions carrying more
    than one sync-wait (CoreV3GenImpl setupSyncWait: "Too many sync wait
    commands").  Tile's scheduler freely assigns several waits to one
    instruction, so (a) wrap _commit_instruction to hoist extra waits onto
    single-wait NOPs on the same engine just before the offender, and
    (b) split the kernel-tail Drain (which collects one wait per DMA queue)
    into a chain of single-wait Drains."""
    import concourse.tile as tile_mod
    import concourse.mybir as mybir
    import bass_rust
    from concourse.vector_clock import ScopedClock

    if getattr(tile_mod.TileContext, "_single_wait_drain_patch", False):
        return

    orig_commit = tile_mod.TileContext._commit_instruction

    def _commit_instruction(self, inst, lazy_reg_writes=True):
        si = getattr(inst, "sync_info", None)
        if (
            si is not None
            and len(si.on_wait) > 1
            and inst.engine != mybir.EngineType.Unassigned
        ):
            waits = list(si.on_wait)
            for w in waits[:-1]:
                nop = mybir.InstNoOp(
                    name=self.nc.get_next_instruction_name(),
                    engine=inst.engine,
                    sync_info=mybir.SyncInfo(on_wait=[w], on_update=[]),
                    bass_nofuse=True,
                )
                orig_commit(self, nop, lazy_reg_writes=False)
            inst.sync_info = mybir.SyncInfo(
                on_wait=[waits[-1]], on_update=list(si.on_update)
            )
        return orig_commit(self, inst, lazy_reg_writes)

    tile_mod.TileContext._commit_instruction = _commit_instruction

    def _drain_and_barrier(self, tick_clock, wait_clock):
        drain_inst = self.nc.sync.drain()
        wait_clock.add_sem_waits(
            drain_inst.ins, ScopedClock({None: tick_clock.global_clock})
        )
        si = drain_inst.ins.sync_info
        if si is not None and len(si.on_wait) > 1:
            waits = list(si.on_wait)
            drain_inst.ins.sync_info = bass_rust.SyncInfo(
                on_wait=[waits[0]], on_update=list(si.on_update)
            )
            for w in waits[1:]:
                extra = self.nc.sync.drain()
                extra.ins.sync_info = bass_rust.SyncInfo(on_wait=[w], on_update=[])
        self.nc.all_engine_barrier()
        popped = self.nc._tile_sem_poison_stack.pop()
        assert popped is self._sem_poison
        self.nc.clear_and_free_semaphores(list(self.sems.allocated().values()))
        self.nc.all_engine_barrier()

    tile_mod.TileContext._drain_and_barrier = _drain_and_barrier
    tile_mod.TileContext._single_wait_drain_patch = True


def _build(descale):
    """descale = 2^-(a+b), baked into the psum->sbuf copies."""
    if descale in _BUILT:
        return _BUILT[descale]
    import concourse.bass as bass
    import concourse.tile as tile
    from concourse import mybir

    _patch_tile_drain()

    nc = bass.Bass("TRN2", debug=False)
    xb_d = nc.dram_tensor(
        "xb", [KTB, P, M_PER_CORE], mybir.dt.bfloat16, kind="ExternalInput"
    ).ap()
    x8_d = nc.dram_tensor(
        "x8", [N8, P, 2, M_PER_CORE], mybir.dt.float8e4, kind="ExternalInput"
    ).ap()
    # n-chunk 0 of w, per-k-tile descriptors
    wb0_d = nc.dram_tensor(
        "wb0", [KTB, P, NW], mybir.dt.bfloat16, kind="ExternalInput"
    ).ap()
    w80_d = nc.dram_tensor(
        "w80", [N8, P, 2, NW], mybir.dt.float8e4, kind="ExternalInput"
    ).ap()
    # n-chunks 1-7: bf16 in two GS-k-tile chunks, fp8 in one block
    wbr_d = nc.dram_tensor(
        "wbr", [NCH - 1, 2, P, GS, NW], mybir.dt.bfloat16, kind="ExternalInput"
    ).ap()
    w8r_d = nc.dram_tensor(
        "w8r", [NCH - 1, P, N8, 2, NW], mybir.dt.float8e4, kind="ExternalInput"
    ).ap()
    y_d = nc.dram_tensor(
        "y", [M_PER_CORE, OUT], mybir.dt.float32, kind="ExternalOutput"
    ).ap()

    with tile.TileContext(nc) as tc:
        with (
            tc.tile_pool(name="xt", bufs=1) as xt_pool,
            tc.tile_pool(name="w0", bufs=1) as w0_pool,
            tc.tile_pool(name="wr", bufs=2) as wr_pool,
            tc.tile_pool(name="y", bufs=8) as y_pool,
            tc.tile_pool(name="ps", bufs=1, space="PSUM") as ps_pool,
        ):
            # Warm the activation engine's function table (1.3us, overlaps
            # the DMA head) so the tail's scalar.mul doesn't pay it.
            warm = xt_pool.tile([P, 2], mybir.dt.float32, name="warm")
            nc.scalar.mul(warm[:], warm[:], 0.0)
            # Warm the PE clock: HAM un-throttles (1.2->2.4GHz) only after
            # ~3.4us of sustained busy, so burn the ~11us DMA head on dummy
            # matmuls over never-written SBUF (values irrelevant; the real
            # groups start with start=True which resets the bank).
            dum_l = xt_pool.tile([P, P], mybir.dt.bfloat16, name="dum_l")
            dum_r = xt_pool.tile([P, NW], mybir.dt.bfloat16, name="dum_r")
            nc.vector.memset(dum_l[:], 0)
            nc.vector.memset(dum_r[:], 0)
            # Small-N dummies (~107ns cold) give fine-grained padding: they
            # keep the PE busy (HAM warm-up) from ~8.5us until the first
            # real operands land ~10.5us, with ~0.1us quantization.
            ps_warm = ps_pool.tile([P, NW], mybir.dt.float32, name="ps0_0")
            for _ in range(60):
                nc.tensor.matmul(
                    ps_warm[:, :64], lhsT=dum_l[:], rhs=dum_r[:, :64],
                    start=True, stop=True,
                )

            xb_sb = xt_pool.tile(
                [P, KTB, M_PER_CORE], mybir.dt.bfloat16, name="xb"
            )
            x8_sb = xt_pool.tile(
                [P, N8, 2, M_PER_CORE], mybir.dt.float8e4, name="x8"
            )
            # Head: interleave x-slice and w0 descriptors in consumption
            # order so MM(kt=0) waits on just the first two transfers.
            wb0_sbs = []
            for kt in range(KTB):
                if kt == 0:
                    # split so the first MM (kt0, m0) waits on 128KB, not 256KB
                    hm = M_PER_CORE // 2
                    nc.sync.dma_start(xb_sb[:, 0, :hm], xb_d[0, :, :hm])
                    w_sb = w0_pool.tile([P, NW], mybir.dt.bfloat16, name="wb0_0")
                    nc.sync.dma_start(w_sb[:], wb0_d[0])
                    nc.sync.dma_start(xb_sb[:, 0, hm:], xb_d[0, :, hm:])
                else:
                    nc.sync.dma_start(xb_sb[:, kt, :], xb_d[kt])
                    w_sb = w0_pool.tile([P, NW], mybir.dt.bfloat16, name=f"wb0_{kt}")
                    nc.sync.dma_start(w_sb[:], wb0_d[kt])
                wb0_sbs.append(w_sb)
            w80_sbs = []
            for blk in range(N8):
                nc.sync.dma_start(x8_sb[:, blk], x8_d[blk])
                w_sb = w0_pool.tile([P, 2, NW], mybir.dt.float8e4, name=f"w80_{blk}")
                nc.sync.dma_start(w_sb[:], w80_d[blk])
                w80_sbs.append(w_sb)
            # Prefetch stream for n-chunks 1-7 (pool slots throttle the
            # lookahead to ~1 chunk).
            wbr_sbs = {}
            w8r_sbs = {}
            for nch in range(1, NCH):
                for h in range(2):
                    w_sb = wr_pool.tile(
                        [P, GS, NW], mybir.dt.bfloat16, name=f"wbr{h}"
                    )
                    nc.sync.dma_start(w_sb[:], wbr_d[nch - 1, h])
                    wbr_sbs[(nch, h)] = w_sb
                w_sb = wr_pool.tile(
                    [P, N8, 2, NW], mybir.dt.float8e4, name="w8r"
                )
                nc.sync.dma_start(w_sb[:], w8r_d[nch - 1])
                w8r_sbs[nch] = w_sb

            def mm_group(ps, mt, wb_of_kt, w8_of_blk):
                msl = slice(mt * P, (mt + 1) * P)
                for kt in range(KTB):
                    nc.tensor.matmul(
                        ps[:],
                        lhsT=xb_sb[:, kt, msl],
                        rhs=wb_of_kt(kt),
                        start=(kt == 0),
                        stop=False,
                    )
                for blk in range(N8):
                    nc.tensor.matmul(
                        ps[:],
                        lhsT=x8_sb[:, blk, :, msl],
                        rhs=w8_of_blk(blk),
                        start=False,
                        stop=(blk == N8 - 1),
                        perf_mode=mybir.MatmulPerfMode.DoubleRow,
                    )

            def emit_out(mt, nch, ps, last):
                """psum -> sbuf (descale by 2^-(a+b)) -> DRAM.  The last
                n-chunk spreads stores across both hw DMA queues (the input
                queue is idle by then) and the final group also splits the
                copy across engines, shrinking the serial tail."""
                y_sb = y_pool.tile([P, NW], mybir.dt.float32, name="y_sb")
                half = NW // 2
                if last:
                    nc.vector.tensor_scalar_mul(y_sb[:, :half], ps[:, :half], descale)
                    nc.scalar.mul(y_sb[:, half:], ps[:, half:], descale)
                else:
                    nc.vector.tensor_scalar_mul(y_sb[:], ps[:], descale)
                nq = 4 if last else 2
                step = NW // nq
                engs = (nc.scalar, nc.sync) if nch == NCH - 1 else (nc.scalar,)
                for s in range(nq):
                    engs[s % len(engs)].dma_start(
                        y_d[
                            mt * P : (mt + 1) * P,
                            nch * NW + s * step : nch * NW + (s + 1) * step,
                        ],
                        y_sb[:, s * step : (s + 1) * step],
                    )

            # n-chunk 0: k-outer over all 8 m-tiles (8 psum banks live) so
            # each w tile feeds 8 back-to-back MMs while the next streams in.
            ps0 = [
                ps_pool.tile([P, NW], mybir.dt.float32, name=f"ps0_{m}")
                for m in range(MT)
            ]
            for kt in range(KTB):
                for mt in range(MT):
                    nc.tensor.matmul(
                        ps0[mt][:],
                        lhsT=xb_sb[:, kt, mt * P : (mt + 1) * P],
                        rhs=wb0_sbs[kt][:],
                        start=(kt == 0),
                        stop=False,
                    )
            for blk in range(N8):
                for mt in range(MT):
                    nc.tensor.matmul(
                        ps0[mt][:],
                        lhsT=x8_sb[:, blk, :, mt * P : (mt + 1) * P],
                        rhs=w80_sbs[blk][:],
                        start=False,
                        stop=(blk == N8 - 1),
                        perf_mode=mybir.MatmulPerfMode.DoubleRow,
                    )
            for mt in range(MT):
                emit_out(mt, 0, ps0[mt], last=False)

            # n-chunks 1-7: m-inner (psum copies stagger across the sweep).
            for nch in range(1, NCH):
                for mt in range(MT):
                    ps = ps_pool.tile([P, NW], mybir.dt.float32, name=f"ps0_{mt}")
                    mm_group(
                        ps,
                        mt,
                        lambda kt, n=nch: wbr_sbs[(n, kt // GS)][:, kt % GS, :],
                        lambda blk, n=nch: w8r_sbs[n][:, blk],
                    )
                    emit_out(
                        mt, nch, ps, last=(nch == NCH - 1 and mt == MT - 1)
                    )
    _BUILT[descale] = nc
    return nc


def _ensure_ntff_hook():
    """bass_utils' trace path imports antenv.axon_hooks, which some images
    lack (trn_boot degrades silently).  Recreate the glue module around the
    libaxon_pjrt.so ctypes hook so trace=True works; no-op if present."""
    import sys
    import types

    try:
        import antenv.axon_hooks  # noqa: F401

        return
    except ImportError:
        pass
    try:
        import antenv

        if "/root/.axon_site" not in sys.path:
            sys.path.insert(0, "/root/.axon_site")
        from trn_agent_boot.trn_boot import _ntff_profile_via_ctypes

        hook = _ntff_profile_via_ctypes("/opt/axon/libaxon_pjrt.so")
        mod = types.ModuleType("antenv.axon_hooks")
        mod._hook = hook
        mod.get_axon_ntff_profile_hook = lambda: mod._hook
        mod.set_axon_ntff_profile_hook = lambda h: setattr(mod, "_hook", h)
        sys.modules["antenv.axon_hooks"] = mod
        antenv.axon_hooks = mod
    except Exception:
        pass  # trace attempt will fall back to trace=False below


def kernel(x, w_q_fp4, w_os_fp4, w_is_fp4, w_t, w_q_fp8, w_s_fp8):
    global LAST_RESULT
    from concourse.bass_utils import run_bass_kernel_spmd

    x = np.asarray(x, dtype=np.float32)
    w_t = np.asarray(w_t, dtype=np.float32)

    bf16 = ml_dtypes.bfloat16
    e4m3 = ml_dtypes.float8_e4m3  # TRN fp8e4: max normal 240

    # power-2 scales placing the fp8-span operands in e4m3 range
    a = float(np.floor(np.log2(FP8_MAX / np.abs(x).max())))
    b = float(np.floor(np.log2(FP8_MAX / np.abs(w_t[KSPLIT:, :]).max())))
    sa, sb = 2.0**a, 2.0**b
    descale = float(2.0 ** (-(a + b)))

    nc = _build(descale)

    def to8(v, s):
        return np.clip(v * s, -FP8_MAX, FP8_MAX).astype(e4m3)

    xt = np.ascontiguousarray(x.T)                     # [IN, TOKENS] fp32
    xb_all = (xt[:KSPLIT] * sa).astype(bf16)           # [KSPLIT, TOKENS]
    x8_all = to8(xt[KSPLIT:], sa)                      # [2*N8*P, TOKENS]

    wsc = w_t * sb
    # n-chunk 0
    wb0 = np.ascontiguousarray(wsc[:KSPLIT, :NW]).astype(bf16).reshape(KTB, P, NW)
    w80 = np.ascontiguousarray(
        to8(wsc[KSPLIT:, :NW], 1.0).reshape(N8, 2, P, NW).transpose(0, 2, 1, 3)
    )
    # n-chunks 1-7
    wbr = np.ascontiguousarray(
        wsc[:KSPLIT, NW:]
        .astype(bf16)
        .reshape(2, GS, P, NCH - 1, NW)
        .transpose(3, 0, 2, 1, 4)
    )
    w8r = np.ascontiguousarray(
        to8(wsc[KSPLIT:, NW:], 1.0)
        .reshape(N8, 2, P, NCH - 1, NW)
        .transpose(3, 2, 0, 1, 4)
    )
    in_maps = []
    for i in range(NCORES):
        msl = slice(i * M_PER_CORE, (i + 1) * M_PER_CORE)
        xb = np.ascontiguousarray(xb_all[:, msl]).reshape(KTB, P, M_PER_CORE)
        x8 = np.ascontiguousarray(
            x8_all[:, msl].reshape(N8, 2, P, M_PER_CORE).transpose(0, 2, 1, 3)
        )
        in_maps.append(
            {"xb": xb, "x8": x8, "wb0": wb0, "w80": w80, "wbr": wbr, "w8r": w8r}
        )
    want_trace = bool(os.environ.get("BASS_TRACE"))
    if want_trace:
        _ensure_ntff_hook()
    res = None
    # retries cover transient device errors (e.g. NRT_EXEC_UNIT_UNRECOVERABLE,
    # observed once and succeeded on retry); the final attempt drops trace in
    # case the profiling path itself is what broke
    for attempt, tr in enumerate((want_trace, want_trace, False)):
        try:
            res = run_bass_kernel_spmd(nc, in_maps, list(range(NCORES)), trace=tr)
            break
        except Exception:
            if attempt == 2:
                raise
    LAST_RESULT = res
    return np.concatenate([res.results[i]["y"] for i in range(NCORES)], axis=0)
